# revision 8
# baseline (speedup 1.0000x reference)
"""Trainium2 Bass kernel for nn_Denoising_ResNet: out = x + conv1x1(box_mean3x3(x)) + b.

Sharding: data-parallel over batch (32 samples -> 4 per core x 8 cores).

Device computes only the conv path y_dev = conv1x1(box_mean3x3(x)) + b in
bf16 I/O (x pre-cast to bf16 on host, y stored as bf16); the residual +x
is added on the host in f32. This halves HBM traffic vs f32 I/O
(16.8MB/core vs 33.6MB) and removes the identity matmul pass from PE.

Per-core layout: 2 "stacks" of 2 samples each -> 128 SBUF partitions
(= 2 samples x 64 channels). Each stack's full bf16 image is loaded to
one SBUF tile by quarter DMAs on the SWDGE (gpsimd) ring, whose
boundaries (34/66/98) align with the 32-row compute chunks' halo spans.
Stack 0's first quarter is split in two so the first W-pass can start
~2us sooner. All constants (w9t | w05t | bias) ride ONE small bf16 DMA
on the sync ring (separate rail; HWDGE DMAs complete serially at ~4us
each, so one combined transfer beats three).

Math decomposition (K=3 edge-clipped box mean, then 1x1 conv):
  - W-direction 3-tap sum: two shifted DVE adds (bf16, 2x DVE mode);
    edge cols folded to 1.5*(2-tap) with one strided tensor_scalar op.
    Emitted in 16-row halves so PE can start on the first half while
    DVE finishes the second.
  - Global /9 of the box mean is folded into the conv weight.
  - H-direction 3-tap sum is FUSED into the 1x1 conv: 3 accumulating
    PE matmuls per 4-row PSUM bank with row-shifted moving operands
    against a block-diagonal [128,128] weight kron(I2, (W/9)^T) in bf16;
    zeroed ws halo rows give the edge-clipped sum at image boundaries.
  - Image-boundary rows get 2 extra in-group matmuls against 0.5x the
    conv weight (-> 1.5x conv total = the edge-clip row count fix).
  - PSUM -> SBUF bf16 copy + bias on the scalar engine (activation
    Identity, bias per partition), one per 16-row (4-bank) PSUM tile.
"""
from contextlib import ExitStack

import numpy as np

import concourse.bass as bass
import concourse.tile as tile
from concourse import bacc, mybir
from concourse.bass_utils import run_bass_kernel_spmd

B, C, H, W = 32, 64, 128, 128
NCORES = 8
PER = B // NCORES  # samples per core
NSTACK = PER // 2  # 2-sample stacks per core
HC = 32  # chunk height (output rows per chunk)
NCHUNK = H // HC
GROUP_ROWS = 4  # rows per matmul accumulation group (512 f32 = 1 bank)
TILE_ROWS = 16  # rows per PSUM tile (4 banks), 4 groups per tile
NTILE = HC // TILE_ROWS
NGROUP = TILE_ROWS // GROUP_ROWS

F32 = mybir.dt.float32
BF16 = mybir.dt.bfloat16


def _build_nc() -> bass.Bass:
    nc = bacc.Bacc("TRN2", debug=False)
    x = nc.dram_tensor("x", [PER * C, H, W], BF16, kind="ExternalInput")
    # consts packed as [w9t | w05t | bias] -> one DMA
    consts = nc.dram_tensor("consts", [2 * C, 2 * 2 * C + 1], BF16, kind="ExternalInput")
    y = nc.dram_tensor("y", [PER * C, H, W], BF16, kind="ExternalOutput")
    xap = x.ap()
    yap = y.ap()
    IDENT_FN = mybir.ActivationFunctionType.Identity

    with ExitStack() as ctx:
        tc = ctx.enter_context(tile.TileContext(nc))
        cpool = ctx.enter_context(tc.tile_pool(name="const", bufs=1))
        ct = cpool.tile([128, 2 * 2 * C + 1], BF16)
        nc.sync.dma_start(out=ct[:], in_=consts.ap()[:, :])
        wt = ct[:, 0 : 2 * C]
        w05 = ct[:, 2 * C : 4 * C]
        bt = ct[:, 4 * C : 4 * C + 1]
        # prewarm the scalar engine's activation table so the first real
        # ACTIVATE doesn't pay the ~1.3us lazy table load.
        warm = cpool.tile([128, 1], BF16)
        nc.scalar.activation(warm[:], bt, mybir.ActivationFunctionType.Identity, bias=bt)

        ppool = ctx.enter_context(tc.tile_pool(name="psum", bufs=2, space="PSUM"))

        xpool = ctx.enter_context(tc.tile_pool(name="xin", bufs=2))
        tpool = ctx.enter_context(tc.tile_pool(name="tmp", bufs=2))
        wpool = ctx.enter_context(tc.tile_pool(name="wsum", bufs=2))
        opool = ctx.enter_context(tc.tile_pool(name="out", bufs=4))

        # all input DMAs up front on the SWDGE ring (xpool holds both
        # stacks); quarter boundaries at 34/66/98 align with the 32-row
        # chunks' halo spans so chunk ci waits only on quarters 0..ci.
        # Stack 0's first quarter is split at rows 10/18 to cut startup.
        xts = []
        for g in range(NSTACK):
            p0 = g * 128
            xt = xpool.tile([128, H, W], BF16)
            xts.append(xt)
            if g == 0:
                qb = [0, 10, TILE_ROWS + 2, HC + 2, 2 * HC + 2, 3 * HC + 2, H]
            else:
                qb = [0, HC + 2, 2 * HC + 2, 3 * HC + 2, H]
            for q in range(len(qb) - 1):
                nc.gpsimd.dma_start(
                    out=xt[:, qb[q] : qb[q + 1], :],
                    in_=xap[p0 : p0 + 128, qb[q] : qb[q + 1], :],
                )

        for g in range(NSTACK):
            p0 = g * 128
            xt = xts[g]
            for ci in range(NCHUNK):
                h0 = ci * HC
                # chunk rows [h0, h0+HC); halo rows clamped at the image edge
                ra = 0 if ci == 0 else h0 - 1       # first xt row read
                rb = h0 + HC if ci == NCHUNK - 1 else h0 + HC + 1

                # W-direction 3-tap on DVE (bf16). tt/ws row r holds
                # image row h0-1+r; out-of-image halo ws rows are zeroed.
                la = ra - (h0 - 1)  # first valid local row (0 or 1)
                lb = rb - (h0 - 1)  # past-last valid local row
                tt = tpool.tile([128, HC + 2, W], BF16)
                ws = wpool.tile([128, HC + 2, W], BF16)
                if ci == 0:
                    nc.vector.memset(ws[:, 0:1, :], 0.0)
                elif ci == NCHUNK - 1:
                    nc.vector.memset(ws[:, HC + 1 : HC + 2, :], 0.0)
                # 16-row halves: PSUM tile tp reads ws rows
                # [16*tp, 16*tp+18), so half boundary at local row 18
                # lets PE tile 0 start while DVE computes the second half.
                # The very first chunk is split finer (10/18) so the first
                # matmul group can start after a 10-row input DMA.
                if g == 0 and ci == 0:
                    segs = ((la, 10), (10, TILE_ROWS + 2), (TILE_ROWS + 2, lb))
                else:
                    segs = ((la, TILE_ROWS + 2), (TILE_ROWS + 2, lb))
                for l0, l1 in segs:
                    r0 = h0 - 1 + l0
                    r1 = h0 - 1 + l1
                    nc.vector.tensor_add(
                        tt[:, l0:l1, 1:W], xt[:, r0:r1, 0 : W - 1], xt[:, r0:r1, 1:W]
                    )
                    nc.vector.tensor_add(
                        ws[:, l0:l1, 1 : W - 1],
                        tt[:, l0:l1, 1 : W - 1],
                        xt[:, r0:r1, 2:W],
                    )
                    # both edge columns in one strided op:
                    # ws[:, :, {0, W-1}] = 1.5 * tt[:, :, {1, W-1}]
                    # chunks after the first go on the (mostly idle)
                    # gpsimd engine to keep DVE off the critical path.
                    eng = nc.vector if (g == 0 and ci == 0) else nc.gpsimd
                    eng.tensor_scalar_mul(
                        ws[:, l0:l1, 0 : W : W - 1],
                        tt[:, l0:l1, 1 : W : W - 2],
                        1.5,
                    )

                ot = opool.tile([128, HC, W], BF16)
                for tp in range(NTILE):
                    ps = ppool.tile([128, TILE_ROWS, W], F32, tag="ps")
                    t0 = tp * TILE_ROWS  # chunk-local first output row of tile
                    # fix_row: tile-local image-boundary row (row-count fix)
                    fix_row = None
                    if ci == 0 and tp == 0:
                        fix_row = 0
                    elif ci == NCHUNK - 1 and tp == NTILE - 1:
                        fix_row = TILE_ROWS - 1
                    # one accumulation group per 4-row bank: 3 H-matmuls
                    # (zero ws halo rows make the boundary rows come out
                    # clipped); for the boundary row 2 extra 0.5x-weight
                    # matmuls (-> 1.5x conv total) close the group.
                    for hp in range(NGROUP):
                        ga, gb = hp * GROUP_ROWS, (hp + 1) * GROUP_ROWS
                        fix_here = fix_row is not None and ga <= fix_row < gb
                        for j, dh in enumerate((-1, 0, 1)):
                            nc.tensor.matmul(
                                ps[:, ga:gb, :],
                                wt,
                                ws[:, 1 + t0 + ga + dh : 1 + t0 + gb + dh, :],
                                start=(j == 0),
                                stop=(j == 2 and not fix_here),
                            )
                        if fix_here:
                            dhs = (0, 1) if fix_row == 0 else (-1, 0)
                            for k, dh in enumerate(dhs):
                                nc.tensor.matmul(
                                    ps[:, fix_row : fix_row + 1, :],
                                    w05,
                                    ws[
                                        :,
                                        1 + t0 + fix_row + dh : 2 + t0 + fix_row + dh,
                                        :,
                                    ],
                                    start=False,
                                    stop=(k == 1),
                                )
                    last_tile = g == NSTACK - 1 and ci == NCHUNK - 1
                    if last_tile and tp == NTILE - 1:
                        # final drain: 8-row activation + DMA pieces so the
                        # kernel's tail is one small transfer, not 16 rows.
                        for s0 in (0, 8):
                            nc.scalar.activation(
                                ot[:, t0 + s0 : t0 + s0 + 8, :],
                                ps[:, s0 : s0 + 8, :],
                                IDENT_FN,
                                bias=bt,
                            )
                            nc.sync.dma_start(
                                out=yap[
                                    p0 : p0 + 128, h0 + t0 + s0 : h0 + t0 + s0 + 8, :
                                ],
                                in_=ot[:, t0 + s0 : t0 + s0 + 8, :],
                            )
                    else:
                        nc.scalar.activation(
                            ot[:, t0 : t0 + TILE_ROWS, :],
                            ps[:],
                            IDENT_FN,
                            bias=bt,
                        )
                        if last_tile:
                            nc.sync.dma_start(
                                out=yap[
                                    p0 : p0 + 128, h0 + t0 : h0 + t0 + TILE_ROWS, :
                                ],
                                in_=ot[:, t0 : t0 + TILE_ROWS, :],
                            )
                if not (g == NSTACK - 1 and ci == NCHUNK - 1):
                    nc.sync.dma_start(
                        out=yap[p0 : p0 + 128, h0 : h0 + HC, :], in_=ot[:]
                    )
    nc.compile()
    return nc


_NC = None


def _get_nc() -> bass.Bass:
    global _NC
    if _NC is None:
        _NC = _build_nc()
    return _NC


def _host_inputs(x: np.ndarray, conv_w: np.ndarray, conv_b: np.ndarray):
    import ml_dtypes

    bf = ml_dtypes.bfloat16
    conv_w = np.asarray(conv_w)
    conv_b = np.asarray(conv_b)
    x = np.asarray(x)
    w9t = np.zeros((2 * C, 2 * C), dtype=np.float32)
    wT = (conv_w.astype(np.float32) / 9.0).T
    w9t[0:C, 0:C] = wT
    w9t[C : 2 * C, C : 2 * C] = wT
    bias2 = np.concatenate([conv_b, conv_b]).reshape(2 * C, 1).astype(np.float32)
    consts = np.zeros((2 * C, 2 * 2 * C + 1), dtype=np.float32)
    consts[:, 0 : 2 * C] = w9t
    consts[:, 2 * C : 4 * C] = w9t * 0.5
    consts[:, 4 * C : 4 * C + 1] = bias2
    consts = consts.astype(bf)
    xb = np.ascontiguousarray(x, dtype=np.float32).astype(bf)
    in_maps = []
    for i in range(NCORES):
        xi = xb[i * PER : (i + 1) * PER].reshape(PER * C, H, W)
        in_maps.append({"x": xi, "consts": consts})
    return in_maps


def _finalize(x: np.ndarray, res) -> np.ndarray:
    """Unshard device results (conv path, bf16) and add the f32 residual x."""
    outs = [
        np.asarray(res.results[i]["y"])
        .reshape(PER, C, H, W)
        .astype(np.float32)
        for i in range(NCORES)
    ]
    conv_part = np.concatenate(outs, axis=0)
    return np.ascontiguousarray(x, dtype=np.float32) + conv_part


def kernel(x: np.ndarray, conv_w: np.ndarray, conv_b: np.ndarray) -> np.ndarray:
    nc = _get_nc()
    in_maps = _host_inputs(x, conv_w, conv_b)
    res = run_bass_kernel_spmd(nc, in_maps, list(range(NCORES)))
    return _finalize(x, res)


# revision 9
# speedup vs baseline: 1.0731x; 1.0731x over previous
"""Trainium2 Bass kernel for nn_Denoising_ResNet: out = x + conv1x1(box_mean3x3(x)) + b.

Sharding: data-parallel over batch (32 samples -> 4 per core x 8 cores).

Device computes only the conv path y_dev = conv1x1(box_mean3x3(x)) + b in
bf16 I/O (x pre-cast to bf16 on host, y stored as bf16); the residual +x
is added on the host in f32. This halves HBM traffic vs f32 I/O
(16.8MB/core vs 33.6MB) and removes the identity matmul pass from PE.

Per-core layout: 2 "stacks" of 2 samples each -> 128 SBUF partitions
(= 2 samples x 64 channels). Each stack's full bf16 image is loaded to
one SBUF tile by quarter DMAs on the SWDGE (gpsimd) ring, whose
boundaries (34/66/98) align with the 32-row compute chunks' halo spans.
Stack 0's first quarter is split in two so the first W-pass can start
~2us sooner. All constants (w9t | w05t | bias) ride ONE small bf16 DMA
on the sync ring (separate rail; HWDGE DMAs complete serially at ~4us
each, so one combined transfer beats three).

Math decomposition (K=3 edge-clipped box mean, then 1x1 conv):
  - W-direction 3-tap sum: two shifted DVE adds (bf16, 2x DVE mode);
    edge cols folded to 1.5*(2-tap) with one strided tensor_scalar op.
    Emitted in 16-row halves so PE can start on the first half while
    DVE finishes the second.
  - Global /9 of the box mean is folded into the conv weight.
  - H-direction 3-tap sum is FUSED into the 1x1 conv: 3 accumulating
    PE matmuls per 4-row PSUM bank with row-shifted moving operands
    against a block-diagonal [128,128] weight kron(I2, (W/9)^T) in bf16;
    zeroed ws halo rows give the edge-clipped sum at image boundaries.
  - Image-boundary rows get 2 extra in-group matmuls against 0.5x the
    conv weight (-> 1.5x conv total = the edge-clip row count fix).
  - PSUM -> SBUF bf16 copy + bias on the scalar engine (activation
    Identity, bias per partition), one per 16-row (4-bank) PSUM tile.
"""
from contextlib import ExitStack

import numpy as np

import concourse.bass as bass
import concourse.tile as tile
from concourse import bacc, mybir
from concourse.bass_utils import run_bass_kernel_spmd

B, C, H, W = 32, 64, 128, 128
NCORES = 8
PER = B // NCORES  # samples per core
NSTACK = PER // 2  # 2-sample stacks per core
HC = 32  # chunk height (output rows per chunk)
NCHUNK = H // HC
GROUP_ROWS = 4  # rows per matmul accumulation group (512 f32 = 1 bank)
TILE_ROWS = 16  # rows per PSUM tile (4 banks), 4 groups per tile
NTILE = HC // TILE_ROWS
NGROUP = TILE_ROWS // GROUP_ROWS

F32 = mybir.dt.float32
BF16 = mybir.dt.bfloat16


def _build_nc() -> bass.Bass:
    nc = bacc.Bacc("TRN2", debug=False)
    x = nc.dram_tensor("x", [PER * C, H, W], BF16, kind="ExternalInput")
    # consts packed as [w9t | w05t | bias] -> one DMA
    consts = nc.dram_tensor("consts", [2 * C, 2 * 2 * C + 1], BF16, kind="ExternalInput")
    y = nc.dram_tensor("y", [PER * C, H, W], BF16, kind="ExternalOutput")
    xap = x.ap()
    yap = y.ap()
    IDENT_FN = mybir.ActivationFunctionType.Identity

    with ExitStack() as ctx:
        tc = ctx.enter_context(tile.TileContext(nc))
        cpool = ctx.enter_context(tc.tile_pool(name="const", bufs=1))
        ct = cpool.tile([128, 2 * 2 * C + 1], BF16)
        nc.sync.dma_start(out=ct[:], in_=consts.ap()[:, :])
        wt = ct[:, 0 : 2 * C]
        w05 = ct[:, 2 * C : 4 * C]
        bt = ct[:, 4 * C : 4 * C + 1]
        # prewarm the scalar engine's activation table so the first real
        # ACTIVATE doesn't pay the ~1.3us lazy table load.
        warm = cpool.tile([128, 1], BF16)
        nc.scalar.activation(warm[:], bt, mybir.ActivationFunctionType.Identity, bias=bt)

        ppool = ctx.enter_context(tc.tile_pool(name="psum", bufs=2, space="PSUM"))

        xpool = ctx.enter_context(tc.tile_pool(name="xin", bufs=2))
        tpool = ctx.enter_context(tc.tile_pool(name="tmp", bufs=2))
        wpool = ctx.enter_context(tc.tile_pool(name="wsum", bufs=2))
        opool = ctx.enter_context(tc.tile_pool(name="out", bufs=4))

        # all input DMAs up front on the SWDGE ring (xpool holds both
        # stacks); quarter boundaries at 34/66/98 align with the 32-row
        # chunks' halo spans so chunk ci waits only on quarters 0..ci.
        # Stack 0's first quarter is split at rows 10/18 to cut startup.
        xts = []
        for g in range(NSTACK):
            p0 = g * 128
            xt = xpool.tile([128, H, W], BF16)
            xts.append(xt)
            if g == 0:
                qb = [0, 10, TILE_ROWS + 2, HC + 2, 2 * HC + 2, 3 * HC + 2, H]
            else:
                qb = [0, HC + 2, 2 * HC + 2, 3 * HC + 2, H]
            for q in range(len(qb) - 1):
                nc.gpsimd.dma_start(
                    out=xt[:, qb[q] : qb[q + 1], :],
                    in_=xap[p0 : p0 + 128, qb[q] : qb[q + 1], :],
                )

        for g in range(NSTACK):
            p0 = g * 128
            xt = xts[g]
            for ci in range(NCHUNK):
                h0 = ci * HC
                # chunk rows [h0, h0+HC); halo rows clamped at the image edge
                ra = 0 if ci == 0 else h0 - 1       # first xt row read
                rb = h0 + HC if ci == NCHUNK - 1 else h0 + HC + 1

                # W-direction 3-tap on DVE (bf16). tt/ws row r holds
                # image row h0-1+r; out-of-image halo ws rows are zeroed.
                la = ra - (h0 - 1)  # first valid local row (0 or 1)
                lb = rb - (h0 - 1)  # past-last valid local row
                tt = tpool.tile([128, HC + 2, W], BF16)
                ws = wpool.tile([128, HC + 2, W], BF16)
                if ci == 0:
                    nc.vector.memset(ws[:, 0:1, :], 0.0)
                elif ci == NCHUNK - 1:
                    nc.vector.memset(ws[:, HC + 1 : HC + 2, :], 0.0)
                # 16-row halves: PSUM tile tp reads ws rows
                # [16*tp, 16*tp+18), so half boundary at local row 18
                # lets PE tile 0 start while DVE computes the second half.
                # The very first chunk is split finer (10/18) so the first
                # matmul group can start after a 10-row input DMA.
                if g == 0 and ci == 0:
                    segs = ((la, 10), (10, TILE_ROWS + 2), (TILE_ROWS + 2, lb))
                else:
                    segs = ((la, TILE_ROWS + 2), (TILE_ROWS + 2, lb))
                for l0, l1 in segs:
                    r0 = h0 - 1 + l0
                    r1 = h0 - 1 + l1
                    nc.vector.tensor_add(
                        tt[:, l0:l1, 1:W], xt[:, r0:r1, 0 : W - 1], xt[:, r0:r1, 1:W]
                    )
                    nc.vector.tensor_add(
                        ws[:, l0:l1, 1 : W - 1],
                        tt[:, l0:l1, 1 : W - 1],
                        xt[:, r0:r1, 2:W],
                    )
                    # both edge columns in one strided op:
                    # ws[:, :, {0, W-1}] = 1.5 * tt[:, :, {1, W-1}]
                    nc.vector.tensor_scalar_mul(
                        ws[:, l0:l1, 0 : W : W - 1],
                        tt[:, l0:l1, 1 : W : W - 2],
                        1.5,
                    )

                ot = opool.tile([128, HC, W], BF16)
                for tp in range(NTILE):
                    ps = ppool.tile([128, TILE_ROWS, W], F32, tag="ps")
                    t0 = tp * TILE_ROWS  # chunk-local first output row of tile
                    # fix_row: tile-local image-boundary row (row-count fix)
                    fix_row = None
                    if ci == 0 and tp == 0:
                        fix_row = 0
                    elif ci == NCHUNK - 1 and tp == NTILE - 1:
                        fix_row = TILE_ROWS - 1
                    # one accumulation group per 4-row bank: 3 H-matmuls
                    # (zero ws halo rows make the boundary rows come out
                    # clipped); for the boundary row 2 extra 0.5x-weight
                    # matmuls (-> 1.5x conv total) close the group.
                    for hp in range(NGROUP):
                        ga, gb = hp * GROUP_ROWS, (hp + 1) * GROUP_ROWS
                        fix_here = fix_row is not None and ga <= fix_row < gb
                        for j, dh in enumerate((-1, 0, 1)):
                            nc.tensor.matmul(
                                ps[:, ga:gb, :],
                                wt,
                                ws[:, 1 + t0 + ga + dh : 1 + t0 + gb + dh, :],
                                start=(j == 0),
                                stop=(j == 2 and not fix_here),
                            )
                        if fix_here:
                            dhs = (0, 1) if fix_row == 0 else (-1, 0)
                            for k, dh in enumerate(dhs):
                                nc.tensor.matmul(
                                    ps[:, fix_row : fix_row + 1, :],
                                    w05,
                                    ws[
                                        :,
                                        1 + t0 + fix_row + dh : 2 + t0 + fix_row + dh,
                                        :,
                                    ],
                                    start=False,
                                    stop=(k == 1),
                                )
                    last_tile = g == NSTACK - 1 and ci == NCHUNK - 1
                    if last_tile and tp == NTILE - 1:
                        # final drain: 8-row activation + DMA pieces so the
                        # kernel's tail is one small transfer, not 16 rows.
                        for s0 in (0, 8):
                            nc.scalar.activation(
                                ot[:, t0 + s0 : t0 + s0 + 8, :],
                                ps[:, s0 : s0 + 8, :],
                                IDENT_FN,
                                bias=bt,
                            )
                            nc.sync.dma_start(
                                out=yap[
                                    p0 : p0 + 128, h0 + t0 + s0 : h0 + t0 + s0 + 8, :
                                ],
                                in_=ot[:, t0 + s0 : t0 + s0 + 8, :],
                            )
                    else:
                        nc.scalar.activation(
                            ot[:, t0 : t0 + TILE_ROWS, :],
                            ps[:],
                            IDENT_FN,
                            bias=bt,
                        )
                        if last_tile:
                            nc.sync.dma_start(
                                out=yap[
                                    p0 : p0 + 128, h0 + t0 : h0 + t0 + TILE_ROWS, :
                                ],
                                in_=ot[:, t0 : t0 + TILE_ROWS, :],
                            )
                if not (g == NSTACK - 1 and ci == NCHUNK - 1):
                    nc.sync.dma_start(
                        out=yap[p0 : p0 + 128, h0 : h0 + HC, :], in_=ot[:]
                    )
    nc.compile()
    return nc


_NC = None


def _get_nc() -> bass.Bass:
    global _NC
    if _NC is None:
        _NC = _build_nc()
    return _NC


def _host_inputs(x: np.ndarray, conv_w: np.ndarray, conv_b: np.ndarray):
    import ml_dtypes

    bf = ml_dtypes.bfloat16
    conv_w = np.asarray(conv_w)
    conv_b = np.asarray(conv_b)
    x = np.asarray(x)
    w9t = np.zeros((2 * C, 2 * C), dtype=np.float32)
    wT = (conv_w.astype(np.float32) / 9.0).T
    w9t[0:C, 0:C] = wT
    w9t[C : 2 * C, C : 2 * C] = wT
    bias2 = np.concatenate([conv_b, conv_b]).reshape(2 * C, 1).astype(np.float32)
    consts = np.zeros((2 * C, 2 * 2 * C + 1), dtype=np.float32)
    consts[:, 0 : 2 * C] = w9t
    consts[:, 2 * C : 4 * C] = w9t * 0.5
    consts[:, 4 * C : 4 * C + 1] = bias2
    consts = consts.astype(bf)
    xb = np.ascontiguousarray(x, dtype=np.float32).astype(bf)
    in_maps = []
    for i in range(NCORES):
        xi = xb[i * PER : (i + 1) * PER].reshape(PER * C, H, W)
        in_maps.append({"x": xi, "consts": consts})
    return in_maps


def _finalize(x: np.ndarray, res) -> np.ndarray:
    """Unshard device results (conv path, bf16) and add the f32 residual x."""
    outs = [
        np.asarray(res.results[i]["y"])
        .reshape(PER, C, H, W)
        .astype(np.float32)
        for i in range(NCORES)
    ]
    conv_part = np.concatenate(outs, axis=0)
    return np.ascontiguousarray(x, dtype=np.float32) + conv_part


def kernel(x: np.ndarray, conv_w: np.ndarray, conv_b: np.ndarray) -> np.ndarray:
    nc = _get_nc()
    in_maps = _host_inputs(x, conv_w, conv_b)
    res = run_bass_kernel_spmd(nc, in_maps, list(range(NCORES)))
    return _finalize(x, res)
